# revision 2
# baseline (speedup 1.0000x reference)
"""Causal self-attention (B=2, T=4096, D=768, H=12) on 8 TRN2 NeuronCores. v2.

Sharding: core c = (batch b = c//4) x (head group g = c%4, 3 heads each).
Each core computes qkv projection for its 3 heads, causal attention, and a
partial output projection (rank-192 slice of W_proj). The host sums the 4
partials per batch and adds b_proj (the "all-reduce" happens at gather time).

v2 changes vs v1 (all compute-structure; same sharding):
  - bf16 storage for x^T, W slices, Q^T/K^T, V, P, O^T and the output
    (PSUM accumulation stays fp32). Halves DMA + SBUF traffic; bf16 matmuls
    run 1 cycle/row at any moving-dim size (fp32r needs N>=256).
  - V is produced directly in natural [token, dk] layout by using x^T chunks
    as the stationary operand (out[tok,dk] = x^T_chunk.T @ W_v_chunk), which
    deletes all PE transposes and the [dk,T] V staging.
  - Diagonal-block trimming: for the 4 k-blocks on the causal diagonal of
    each 512-token q-tile only the q >= k columns are computed/exp'd/
    streamed (packed score layout), and the triangular [128,128] sub-block
    is masked by a DVE multiply with a constant triangle (was: gpsimd
    affine_select over the full block).
  - Projection K=64 tail matmuls run pairwise-concurrent in disjoint PE row
    groups (ot2/wp1 duplicated to partitions 64:128).
"""

import numpy as np

from concourse import bacc, mybir, tile
from concourse.masks import make_identity as masks_make_identity
from concourse.bass_utils import run_bass_kernel_spmd

F32 = mybir.dt.float32
BF16 = mybir.dt.bfloat16
EXP = mybir.ActivationFunctionType.Exp
NP_BF16 = mybir.dt.np(mybir.dt.bfloat16)

B, T, D = 2, 4096, 768
H, DK = 12, 64
HPC = 3                  # heads per core
MQ = HPC * DK            # 192 cols per q/k/v slice
MS = 3 * MQ              # 576 total W_qkv slice cols
SCALE = 1.0 / 8.0        # 1/sqrt(DK)

TCH = 512                # token chunk (= q-tile width)
NTCH = T // TCH          # 8
KB = 128                 # k block size
VAW = 3 * (DK + 1)       # 195 cols per k-block in the V-augmented tile

_cached = {}

# bisection flags (set before first _get_nc() call)
TRIM = True        # diagonal-block trimming (partial-range PSUM accumulation)
VNAT = True        # natural-layout V via x^T-as-stationary
PAIR_PROJ = True   # K=64 projection matmuls in paired row groups
UMASK = True       # mask diag triangle via PE-side U accumulation (else DVE mult)

# test introspection: last BassKernelResults
last_results = None


def _build_nc(repeats=1):
    nc = bacc.Bacc("TRN2", target_bir_lowering=False)

    x_d = nc.dram_tensor("x", [D, T], BF16, kind="ExternalInput")
    wq_d = nc.dram_tensor("wq", [D, MS], BF16, kind="ExternalInput")
    bq_d = nc.dram_tensor("bq", [MS], F32, kind="ExternalInput")
    wp_d = nc.dram_tensor("wp", [MQ, D], BF16, kind="ExternalInput")
    out_d = nc.dram_tensor("out", [T, D], BF16, kind="ExternalOutput")

    with tile.TileContext(nc) as tc:
        with (
            tc.tile_pool(name="sbf", bufs=1) as P,
            tc.tile_pool(name="ps", bufs=1, space="PSUM") as PS,
        ):
            for _rep in range(repeats):
                _emit(nc, tc, P, PS, x_d, wq_d, bq_d, wp_d, out_d)

    nc.compile()
    return nc


def _emit(nc, tc, P, PS, x_d, wq_d, bq_d, wp_d, out_d):
    # ---------------- persistent tiles + constant/weight loads ----------------
    w_sb = []
    for c in range(6):
        w = P.tile([128, MS], BF16, tag=f"w{c}", name=f"w{c}")
        w_sb.append(w)
    # interleave weight-slice and round-0 x^T DMAs so qkv matmul c starts
    # as soon as (w_c, xt0_c) have both landed
    xt0 = P.tile([128, 6 * TCH], BF16, tag="xt", bufs=2, name="xt0")
    for c in range(6):
        nc.sync.dma_start(w_sb[c][:], wq_d[c * 128:(c + 1) * 128, :])
        nc.sync.dma_start(xt0[:, c * TCH:(c + 1) * TCH], x_d[c * 128:(c + 1) * 128, 0:TCH])

    # q/k biases: col m holds bq[m*128:(m+1)*128] (per-partition scalars)
    bias_sb = P.tile([128, 3], F32, tag="bias")
    for m in range(3):
        nc.sync.dma_start(
            bias_sb[:, m:m + 1],
            bq_d[m * 128: m * 128 + 128].unsqueeze(-1),
        )
    # v bias: [1,192] broadcast to all partitions (free-dim bias for natural V)
    bias_v1 = P.tile([1, MQ], F32, tag="biasv1")
    nc.sync.dma_start(bias_v1[:], bq_d[2 * MQ: 3 * MQ].unsqueeze(0))
    bias_vb = P.tile([128, MQ], F32, tag="biasvb")
    nc.gpsimd.partition_broadcast(bias_vb[:], bias_v1[:])

    wp0 = P.tile([128, D], BF16, tag="wp0")
    nc.sync.dma_start(wp0[:], wp_d[0:128, :])
    # wp1: rows 0:64 and 64:128 both hold wp_d[128:192] (row-group pairing)
    wp1 = P.tile([128, D], BF16, tag="wp1")
    nc.sync.dma_start(wp1[0:64, :], wp_d[128:192, :])
    nc.sync.dma_start(wp1[64:128, :], wp_d[128:192, :])

    # constant triangle: tri[p, f] = 1 where f >= p else 0 (keep q >= k).
    # Built in f32 on gpsimd (Q7 ucode is fp32-only), converted on DVE.
    tri_f = P.tile([128, 128], F32, tag="trif")
    nc.gpsimd.memset(tri_f[:], 1.0)
    nc.gpsimd.affine_select(
        out=tri_f[:], in_=tri_f[:],
        compare_op=mybir.AluOpType.is_ge,
        fill=0.0, base=0,
        pattern=[[1, 128]], channel_multiplier=-1,
    )
    tri = P.tile([128, 128], BF16, tag="tri")
    nc.vector.tensor_copy(tri[:], tri_f[:])

    # additive mask U[p,f] = 0 where f >= p else -1e38, and a bf16 identity:
    # accumulating ident.T @ U onto a diagonal score block pre-masks it, so
    # exp produces exact zeros with no post-exp DVE work.
    um_f = P.tile([128, 128], F32, tag="umf")
    nc.gpsimd.memset(um_f[:], 0.0)
    nc.gpsimd.affine_select(
        out=um_f[:], in_=um_f[:],
        compare_op=mybir.AluOpType.is_ge,
        fill=-1e38, base=0,
        pattern=[[1, 128]], channel_multiplier=-1,
    )
    um = P.tile([128, 128], BF16, tag="um")
    nc.vector.tensor_copy(um[:], um_f[:])
    idb_f = P.tile([128, 128], F32, tag="idbf")
    masks_make_identity(nc, idb_f[:])
    idb = P.tile([128, 128], BF16, tag="idb")
    nc.vector.tensor_copy(idb[:], idb_f[:])

    # untrimmed fallback: per-r full-width masks (mask_r[p,f]=1 iff f>=p+128r)
    rmasks = None
    if not TRIM:
        rmasks = []
        for r in range(4):
            mf = P.tile([128, TCH], F32, tag=f"rmf{r}", bufs=1, name=f"rmf{r}")
            nc.gpsimd.memset(mf[:], 1.0)
            nc.gpsimd.affine_select(
                out=mf[:], in_=mf[:],
                compare_op=mybir.AluOpType.is_ge,
                fill=0.0, base=-128 * r,
                pattern=[[1, TCH]], channel_multiplier=-1,
            )
            mb_ = P.tile([128, TCH], BF16, tag=f"rmb{r}", bufs=1, name=f"rmb{r}")
            nc.vector.tensor_copy(mb_[:], mf[:])
            rmasks.append(mb_)

    # Q^T/K^T packed: tQ01/tK01 rows 0-63 = head0, rows 64-127 = head1.
    # tQK2: rows 0-63 = {q_h2 cols 0..T, k_h2 cols T..2T}, rows 64-127 dup
    # (so consecutive h2 score matmuls run in disjoint PE row groups).
    tQ01 = P.tile([128, T], BF16, tag="tq01")
    tK01 = P.tile([128, T], BF16, tag="tk01")
    tQK2 = P.tile([128, 2 * T], BF16, tag="tqk2")

    # V augmented, natural layout: per k-block kb, cols kb*195 + h*65 + (0..63)
    # hold V rows, col kb*195 + h*65 + 64 holds ones (softmax denominator).
    vaug = P.tile([128, 32 * VAW], BF16, tag="vaug")
    ones_col = P.tile([128, 1], F32, tag="ones")
    nc.gpsimd.memset(ones_col[:], 1.0)
    vkb = vaug[:].rearrange("p (kb c) -> p kb c", c=VAW)
    for h in range(3):
        # ones column of every k-block for head h: one strided broadcast copy
        nc.vector.tensor_copy(
            vkb[:, :, h * 65 + 64: h * 65 + 65],
            ones_col[:].broadcast_to([128, 32]).unsqueeze(-1),
        )

    _attention = _attention_factory(
        nc, tc, P, PS, out_d, wp0, wp1, tQ01, tK01, tQK2, vaug, tri, rmasks,
        um, idb,
    )

    # ---------------- main loop over 512-token rounds ----------------
    # Software pipeline: phase-1 of round t+1 is emitted mid-attention of
    # round t (after the first j), so its PE-heavy matmuls fill the
    # Act-bound bubbles of the attention phase.
    def phase1_units(t_):
        """Emission units for round t_'s qkv work: [m0, m1, m2, v0..v3]."""
        tcols = slice(t_ * TCH, (t_ + 1) * TCH)
        if t_ == 0:
            xt = xt0
        else:
            xt = P.tile([128, 6 * TCH], BF16, tag="xt", bufs=2, name="xt")
            for c in range(6):
                nc.sync.dma_start(
                    xt[:, c * TCH:(c + 1) * TCH],
                    x_d[c * 128:(c + 1) * 128, t_ * TCH:(t_ + 1) * TCH],
                )
        units = []

        def emit_m(m):
            acc = PS.tile([128, TCH], F32, tag="o", bufs=4, name="acc")
            for c in range(6):
                nc.tensor.matmul(
                    acc[:, :],
                    w_sb[c][:, m * 128: m * 128 + 128],
                    xt[:, c * TCH:(c + 1) * TCH],
                    start=(c == 0), stop=(c == 5),
                )
            if m == 0:      # q_h0 | q_h1
                nc.vector.tensor_scalar_add(
                    tQ01[:, tcols], acc[:, :], bias_sb[:, 0:1])
            elif m == 1:    # q_h2 | k_h0
                nc.vector.tensor_scalar_add(
                    tQK2[0:64, tcols], acc[0:64, :], bias_sb[0:64, 1:2])
                nc.vector.tensor_scalar_add(
                    tQK2[64:128, tcols], acc[0:64, :], bias_sb[0:64, 1:2])
                nc.vector.tensor_scalar_add(
                    tK01[0:64, tcols], acc[64:128, :], bias_sb[64:128, 1:2])
            else:           # k_h1 | k_h2
                nc.vector.tensor_scalar_add(
                    tK01[64:128, tcols], acc[0:64, :], bias_sb[0:64, 2:3])
                kcols = slice(T + t_ * TCH, T + (t_ + 1) * TCH)
                nc.vector.tensor_scalar_add(
                    tQK2[0:64, kcols], acc[64:128, :], bias_sb[64:128, 2:3])
                nc.vector.tensor_scalar_add(
                    tQK2[64:128, kcols], acc[64:128, :], bias_sb[64:128, 2:3])

        def emit_v(r):
            kb = 4 * t_ + r
            accv = PS.tile([128, TCH], F32, tag="o", bufs=4, name="accv")
            for c in range(6):
                nc.tensor.matmul(
                    accv[:, 0:MQ],
                    xt[:, c * TCH + r * 128: c * TCH + (r + 1) * 128],
                    w_sb[c][:, 2 * MQ: 3 * MQ],
                    start=(c == 0), stop=(c == 5),
                )
            nc.vector.tensor_add(
                vkb[:, kb, :].rearrange("p (h c) -> p h c", c=65)[:, :, 0:DK],
                accv[:, 0:MQ].rearrange("p (h c) -> p h c", c=DK),
                bias_vb[:].rearrange("p (h c) -> p h c", c=DK),
            )

        for m in range(3):
            units.append(lambda m=m: emit_m(m))
        for r in range(4) if VNAT else []:
            units.append(lambda r=r: emit_v(r))
        return units

        if not VNAT:
            for r in range(4):
                kb = 4 * t_ + r
                nc.vector.tensor_copy(
                    vkb[:, kb, :].rearrange("p (h c) -> p h c", c=65)[:, :, 0:DK],
                    xt[:, r * 128: r * 128 + MQ].rearrange("p (h c) -> p h c", c=DK),
                )

    for t_ in range(NTCH):
        for u in phase1_units(t_):
            u()
        _attention(t_)

    return


def _attention_factory(nc, tc, P, PS, out_d, wp0, wp1, tQ01, tK01, tQK2, vaug, tri, rmasks,
                       um, idb):

    def _attention(t_, units=()):
        units = list(units)
        tcols = slice(t_ * TCH, (t_ + 1) * TCH)
        qt = t_
        nkb = 4 * (qt + 1)

        oacc = []
        for h in range(3):
            o = PS.tile([65, TCH], F32, tag="o", bufs=4, name=f"oacc{h}")
            oacc.append(o)

        def geom(kb):
            """(roff, n) for block kb: q columns [roff, 512) are computed."""
            r = kb - 4 * qt
            if TRIM and r >= 0:
                return 128 * r, TCH - 128 * r
            return 0, TCH

        def pv(h, kb, p_ap, roff, n):
            nc.tensor.matmul(
                oacc[h][:, roff:TCH],
                vaug[:, kb * VAW + h * 65: kb * VAW + (h + 1) * 65],
                p_ap,
                start=(kb == 0),
                stop=(kb == nkb - 1),
                skip_group_check=TRIM,
            )

        # heads 0/1: same k-block in complementary PE row groups
        def exp_regions(dst, src, g):
            # blocks live at fixed offsets (0, TCH); exp only the valid spans
            if g[0][1] == TCH and g[1][1] == TCH:
                nc.scalar.activation(dst[:, 0:2 * TCH], src[:, 0:2 * TCH],
                                     EXP, scale=SCALE)
            else:
                nc.scalar.activation(dst[:, 0:g[0][1]], src[:, 0:g[0][1]],
                                     EXP, scale=SCALE)
                nc.scalar.activation(dst[:, TCH:TCH + g[1][1]],
                                     src[:, TCH:TCH + g[1][1]],
                                     EXP, scale=SCALE)

        def mask_p(pt, kb, i, g, offs):
            if kb < 4 * qt:
                return
            if TRIM:
                if not UMASK:
                    ms = slice(offs[i], offs[i] + KB)
                    nc.vector.tensor_mul(pt[:, ms], pt[:, ms], tri[:])
            else:
                ms = slice(offs[i], offs[i] + TCH)
                nc.vector.tensor_mul(pt[:, ms], pt[:, ms],
                                     rmasks[kb - 4 * qt][:])

        # all 3 heads per j: h0/h1 paired on row groups, h2 paired via dup
        for j in range(nkb // 2):
            kbs = (2 * j, 2 * j + 1)
            g = [geom(kb) for kb in kbs]
            offs = (0, TCH)
            sA = PS.tile([128, 2 * TCH], F32, tag="s", bufs=2, name="sA")
            for i, kb in enumerate(kbs):
                roff, n = g[i]
                krange = slice(kb * KB, (kb + 1) * KB)
                qs = slice(t_ * TCH + roff, (t_ + 1) * TCH)
                cs = slice(offs[i], offs[i] + n)
                diag = TRIM and UMASK and kb >= 4 * qt
                nc.tensor.matmul(
                    sA[:, cs], tK01[0:64, krange], tQ01[0:64, qs],
                    start=True, stop=True,
                )
                if diag:
                    ms = slice(offs[i], offs[i] + KB)
                    nc.tensor.matmul(
                        sA[:, ms], idb[:], um[:],
                        start=False, stop=True,
                        skip_group_check=True,
                    )
            pA = P.tile([128, 2 * TCH], BF16, tag="pt", bufs=6, name="pA")
            exp_regions(pA, sA, g)
            sB = PS.tile([128, 2 * TCH], F32, tag="s", bufs=2, name="sB")
            for i, kb in enumerate(kbs):
                roff, n = g[i]
                krange = slice(kb * KB, (kb + 1) * KB)
                qs = slice(t_ * TCH + roff, (t_ + 1) * TCH)
                cs = slice(offs[i], offs[i] + n)
                diag = TRIM and UMASK and kb >= 4 * qt
                nc.tensor.matmul(
                    sB[:, cs], tK01[64:128, krange], tQ01[64:128, qs],
                    start=True, stop=True,
                )
                if diag:
                    ms = slice(offs[i], offs[i] + KB)
                    nc.tensor.matmul(
                        sB[:, ms], idb[:], um[:],
                        start=False, stop=True,
                        skip_group_check=True,
                    )
            pB = P.tile([128, 2 * TCH], BF16, tag="pt", bufs=6, name="pB")
            exp_regions(pB, sB, g)
            sC = PS.tile([128, 2 * TCH], F32, tag="s", bufs=2, name="sC")
            for i, kb in enumerate(kbs):
                roff, n = g[i]
                rg = slice(64 * i, 64 * i + 64)
                krange = slice(T + kb * KB, T + (kb + 1) * KB)
                cs = slice(offs[i], offs[i] + n)
                diag = TRIM and UMASK and kb >= 4 * qt
                nc.tensor.matmul(
                    sC[:, cs], tQK2[rg, krange],
                    tQK2[rg, t_ * TCH + roff:(t_ + 1) * TCH],
                    start=True, stop=True,
                )
                if diag:
                    ms = slice(offs[i], offs[i] + KB)
                    nc.tensor.matmul(
                        sC[:, ms], idb[:], um[:],
                        start=False, stop=True,
                        skip_group_check=True,
                    )
            pC = P.tile([128, 2 * TCH], BF16, tag="pt", bufs=6, name="pC")
            exp_regions(pC, sC, g)
            for i, kb in enumerate(kbs):
                roff, n = g[i]
                cs = slice(offs[i], offs[i] + n)
                mask_p(pA, kb, i, g, offs)
                pv(0, kb, pA[:, cs], roff, n)
                mask_p(pB, kb, i, g, offs)
                pv(1, kb, pB[:, cs], roff, n)
                mask_p(pC, kb, i, g, offs)
                pv(2, kb, pC[:, cs], roff, n)


        # ---- normalize: O^T[d,q] * (1/sum[q]) ----
        ot01 = P.tile([128, TCH], BF16, tag="ot01", bufs=2, name="ot01")
        ot2 = P.tile([128, TCH], BF16, tag="ot2", bufs=2, name="ot2")
        for h in range(3):
            rc = P.tile([1, TCH], F32, tag="rc", bufs=3, name="rc")
            nc.vector.reciprocal(rc[:], oacc[h][64:65, :])
            rb = P.tile([64, TCH], F32, tag="rb", bufs=3, name="rb")
            nc.gpsimd.partition_broadcast(rb[:], rc[:])
            if h == 0:
                nc.vector.tensor_mul(ot01[0:64, :], oacc[h][0:64, :], rb[:])
            elif h == 1:
                nc.vector.tensor_mul(ot01[64:128, :], oacc[h][0:64, :], rb[:])
            else:
                nc.vector.tensor_mul(ot2[0:64, :], oacc[h][0:64, :], rb[:])
                nc.vector.tensor_mul(ot2[64:128, :], oacc[h][0:64, :], rb[:])

        # ---- phase 3: partial projection y = O^T.T @ W_proj_slice ----
        # 128-token blocks in pairs; the K=64 tail matmuls of a pair run
        # concurrently in disjoint PE row groups.
        for rp in range(2):
            pps = []
            for i in range(2):
                r = 2 * rp + i
                pp = PS.tile([128, 2 * TCH], F32, tag="s", bufs=2, name="pps")
                tcl = slice(r * 128, (r + 1) * 128)
                for ns in (slice(0, 512), slice(512, D)):
                    nc.tensor.matmul(
                        pp[:, ns], ot01[:, tcl], wp0[:, ns],
                        start=True, stop=False,
                    )
                pps.append(pp)
            for ns in (slice(0, 512), slice(512, D)):
                for i in range(2):
                    r = 2 * rp + i
                    rg = slice(64 * i, 64 * i + 64) if PAIR_PROJ else slice(0, 64)
                    tcl = slice(r * 128, (r + 1) * 128)
                    nc.tensor.matmul(
                        pps[i][:, ns], ot2[rg, tcl], wp1[rg, ns],
                        start=False, stop=True,
                    )
            for i in range(2):
                r = 2 * rp + i
                yo = P.tile([128, D], BF16, tag="yo", bufs=3, name="yo")
                nc.vector.tensor_copy(yo[:], pps[i][:, 0:D])
                row0 = qt * TCH + r * 128
                nc.sync.dma_start(out_d[row0:row0 + 128, :], yo[:])

    return _attention


def _get_nc():
    if "nc" not in _cached:
        _cached["nc"] = _build_nc()
    return _cached["nc"]


def _make_in_maps(x, W_qkv, b_qkv, W_proj):
    in_maps = []
    for c in range(8):
        b, g = c // 4, c % 4
        lo, hi = g * MQ, (g + 1) * MQ
        cols = np.r_[lo:hi, D + lo: D + hi, 2 * D + lo: 2 * D + hi]
        in_maps.append({
            "x": np.ascontiguousarray(x[b].T).astype(NP_BF16),
            "wq": np.ascontiguousarray(W_qkv[:, cols]).astype(NP_BF16),
            "bq": np.ascontiguousarray(b_qkv[cols]),
            "wp": np.ascontiguousarray(W_proj[lo:hi, :]).astype(NP_BF16),
        })
    return in_maps


def kernel(x, W_qkv, b_qkv, W_proj, b_proj):
    global last_results
    x = np.asarray(x, dtype=np.float32)
    W_qkv = np.asarray(W_qkv, dtype=np.float32)
    b_qkv = np.asarray(b_qkv, dtype=np.float32)
    W_proj = np.asarray(W_proj, dtype=np.float32)
    b_proj = np.asarray(b_proj, dtype=np.float32)

    nc = _get_nc()
    in_maps = _make_in_maps(x, W_qkv, b_qkv, W_proj)

    res = run_bass_kernel_spmd(nc, in_maps, core_ids=list(range(8)))
    last_results = res

    y = np.zeros((B, T, D), dtype=np.float32)
    for c in range(8):
        y[c // 4] += np.asarray(res.results[c]["out"]).astype(np.float32)
    y += b_proj[None, None, :]
    return y


def _pjrt_callable(nc):
    """jit-compiled shard_map callable executing nc's NEFF once on 8 cores."""
    import jax
    from jax.experimental.shard_map import shard_map
    from jax.sharding import Mesh, NamedSharding, PartitionSpec

    from concourse import bass2jax

    bass2jax.install_neuronx_cc_hook()
    partition_name = (
        nc.partition_id_tensor.name if nc.partition_id_tensor else None
    )
    in_names, out_names, out_avals = [], [], []
    for alloc in nc.m.functions[0].allocations:
        if not isinstance(alloc, mybir.MemoryLocationSet):
            continue
        name = alloc.memorylocations[0].name
        if alloc.kind == "ExternalInput":
            if name != partition_name:
                in_names.append(name)
        elif alloc.kind == "ExternalOutput":
            out_names.append(name)
            out_avals.append(
                jax.core.ShapedArray(
                    tuple(alloc.tensor_shape), mybir.dt.np(alloc.dtype)
                )
            )
    all_names = in_names + out_names + ([partition_name] if partition_name else [])

    def _body(*args):
        operands = list(args)
        if partition_name is not None:
            operands.append(bass2jax.partition_id_tensor())
        outs = bass2jax._bass_exec_p.bind(
            *operands,
            out_avals=tuple(out_avals),
            in_names=tuple(all_names),
            out_names=tuple(out_names),
            lowering_input_output_aliases=(),
            sim_require_finite=True,
            sim_require_nnan=True,
            nc=nc,
        )
        return tuple(outs)

    devices = jax.devices()[:8]
    mesh = Mesh(np.asarray(devices), ("core",))
    spec = NamedSharding(mesh, PartitionSpec("core"))
    f = jax.jit(
        shard_map(
            _body, mesh=mesh,
            in_specs=(PartitionSpec("core"),) * (len(in_names) + len(out_names)),
            out_specs=(PartitionSpec("core"),) * len(out_names),
            check_rep=False,
        ),
        keep_unused=True,
    )
    return f, in_names, out_avals, spec


def bench(inputs, n_iters=20, repeats=6, batch=8):
    """Per-execution device time.

    Two NEFFs (1x and `repeats`x the kernel body) are each dispatched
    `batch` times asynchronously and blocked on once, so the axon RTT
    jitter is amortized over batch*(repeats-1) kernel executions:
        T = (wall_R - wall_1) / (batch * (repeats - 1))
    Median over n_iters of interleaved pairs.
    """
    import time as _time

    import jax

    in_maps = _make_in_maps(
        np.asarray(inputs["x"], np.float32),
        np.asarray(inputs["W_qkv"], np.float32),
        np.asarray(inputs["b_qkv"], np.float32),
        np.asarray(inputs["W_proj"], np.float32),
    )

    def _prep(nc):
        f, in_names, out_avals, spec = _pjrt_callable(nc)
        concat_in = [
            np.concatenate([np.asarray(m[name]) for m in in_maps], axis=0)
            for name in in_names
        ]
        concat_zero = [
            np.zeros((8 * a.shape[0], *a.shape[1:]), a.dtype) for a in out_avals
        ]
        args = [jax.device_put(a, spec) for a in concat_in + concat_zero]
        jax.block_until_ready(f(*args))  # compile + warm
        return f, args

    f1, args1 = _prep(_get_nc())
    if "ncR" not in _cached:
        _cached["ncR"] = _build_nc(repeats=repeats)
    fR, argsR = _prep(_cached["ncR"])

    def _run_batch(f, args, nb):
        outs = [f(*args) for _ in range(nb)]
        jax.block_until_ready(outs)

    _run_batch(f1, args1, batch)
    _run_batch(fR, argsR, batch)

    diffs = []
    for _ in range(n_iters):
        t0 = _time.perf_counter()
        _run_batch(f1, args1, batch)
        t1 = _time.perf_counter()
        _run_batch(fR, argsR, batch)
        t2 = _time.perf_counter()
        diffs.append((t2 - t1) - (t1 - t0))
    diffs = np.asarray(diffs)
    denom = batch * (repeats - 1)
    med = float(np.median(diffs))
    mad = float(np.median(np.abs(diffs - med)))
    t = max(med, 0.0) / denom
    print(f"  [bench] paired median T = {t*1e6:.1f} us (+-{mad/denom*1e6:.2f})")
    return t * 1e9
